# revision 32
# baseline (speedup 1.0000x reference)
# Trainium2 Bass kernel for nn_Encoder (6-layer conv-attention encoder).
# Sharding: 4 batch groups x 2-way sequence split. Each core owns one half of
# one batch element's sequence (512 columns, no halo in the residual). k/v are
# computed for the own half only; the partner half arrives via one fused
# AllReduce(sum) + local subtract per layer (partner = sum - own). The FFN's
# conv halo (4 columns of post-LN1 x1b) is exchanged the same way mid-layer.
# All SBUF data is fp16 (2x/4x DVE modes, better precision than bf16); PSUM
# accumulation stays f32.
import sys
sys.path.insert(0, '/opt/trn_rl_repo')
import numpy as np

from concourse import bacc, tile, mybir
import concourse.bass as bass
from concourse.bass_utils import run_bass_kernel_spmd

B, C, T = 4, 512, 1024
F, KW, L, H = 2048, 3, 6, 8
KC, DR = 64, 32
TO = 512                   # own cols
NC8 = 8
F16 = mybir.dt.float16
F32 = mybir.dt.float32
AF = mybir.ActivationFunctionType
ALU = mybir.AluOpType
EPS = 1e-4
P = 128

_CACHE = {}
TRACE = False
LAST_RESULT = None


def _emit(nc, tc, d, flags, n_layers=L, do_gather=True):
    (has_bv, ln1_aff, ln2_aff) = flags
    from contextlib import ExitStack
    ctx = ExitStack()

    def pool(name, bufs, space="SBUF"):
        return ctx.enter_context(tc.tile_pool(name=name, bufs=bufs, space=space))

    pers = pool("pers", 1)
    dram = pool("dram", 1, space="DRAM")

    p_kraw = pool("kraw", 2)
    p_qraw = pool("qraw", 2)
    p_tmp = pool("ropetmp", 3)
    p_shuf = pool("shuf", 2)
    p_pt = pool("pt", 8)
    p_rbc = pool("rbc", 4)
    p_resid = pool("resid", 10)
    p_lntmp = pool("lntmp", 6)
    p_lndx = pool("lndx", 4)
    p_sq = pool("sq", 8)
    p_rstd = pool("rstd", 2)
    p_ht = pool("ht", 2)
    p_hm = pool("hm", 4)
    p_wq = pool("wq", 2)
    p_wk = pool("wk", 2)
    p_wv = pool("wv", 2)
    p_wo = pool("wo", 2)
    p_w1 = pool("w1", 3)
    p_w2 = pool("w2", 3)
    p_par = pool("par", 2)
    p_out = pool("outp", 1)
    p_halo = pool("halo", 4)
    p_ksum = pool("ksum", 4)

    x_t = [pers.tile([P, TO], F16, tag=f"x{m}", name=f"x{m}") for m in range(4)]
    kr_t = [pers.tile([P, T], F16, tag=f"kr{m}", name=f"kr{m}") for m in range(4)]
    q_t = [pers.tile([P, TO], F16, tag=f"q{m}", name=f"q{m}") for m in range(4)]
    vt_t = [pers.tile([P, 520], F16, tag=f"vt{j}", name=f"vt{j}") for j in range(8)]
    onorm_t = [pers.tile([P, TO], F16, tag=f"on{i}", name=f"on{i}") for i in range(4)]
    rz_t = [pers.tile([65, 520], F16, tag=f"rz{s}", name=f"rz{s}") for s in range(2)]
    ones64 = pers.tile([65, 64], F16, tag="ones64", name="ones64")
    x1b_t = [pers.tile([P, 516], F16, tag=f"x1b{m}", name=f"x1b{m}") for m in range(4)]
    SWAP_MASK = list(range(16, 32)) + list(range(0, 16))
    cos_k = pers.tile([P, TO], F16, tag="cosk", name="cosk")
    sin_k = pers.tile([P, TO], F16, tag="sink", name="sink")
    maskh = pers.tile([P, 514], F16, tag="maskh", name="maskh")
    onesm = pers.tile([P, 256], F16, tag="onesm", name="onesm")
    eps_sb = pers.tile([P, 1], F32, tag="eps", name="eps")
    hcoef = pers.tile([P, 4], F32, tag="hcoef", name="hcoef")

    dma = nc.sync.dma_start
    for name, t in [("cos_k_d", cos_k), ("sin_k_d", sin_k), ("maskh_d", maskh),
                    ("ones_d", onesm), ("hcoef_d", hcoef)]:
        dma(t[:, :], d[name][:, :])
    for m in range(4):
        dma(x_t[m][:, :], d["x0_d"][m * P:(m + 1) * P, :])
    nc.vector.memset(eps_sb[:, :], EPS)
    for s in range(2):
        nc.vector.memset(rz_t[s][0:64, :], 0.0)
    nc.vector.memset(ones64[0:64, :], 0.0)
    nc.vector.memset(ones64[64:65, :], 1.0)

    mm = nc.tensor.matmul

    def ln(xr_l, chunks, par, affcols, out_l, psum_pool, ptag, out_off=0):
        """Channel LayerNorm over 512 columns. xr_l: 4 [P, 512] f16 tiles.
        Writes out_l tiles at column offset out_off. chunks: column ranges
        processed as independent pipelined chains."""
        sum_ps = psum_pool.tile([P, 1024], F32, tag=ptag, name=ptag)
        sq_ps = psum_pool.tile([P, 1024], F32, tag=ptag, name=ptag)
        for (o, n) in chunks:
            for kk in range(4):
                mm(sum_ps[:, o:o + n], onesm[:, 0:128], xr_l[kk][:, o:o + n],
                   start=(kk == 0), stop=(kk == 3), skip_group_check=True)
            sq_l = []
            for kk in range(4):
                sq = p_sq.tile([P, TO], F16, tag="sq", name="sq")
                nc.vector.tensor_mul(sq[:, o:o + n], xr_l[kk][:, o:o + n],
                                     xr_l[kk][:, o:o + n])
                sq_l.append(sq)
            for kk in range(4):
                mm(sq_ps[:, o:o + n], onesm[:, 128:256], sq_l[kk][:, o:o + n],
                   start=(kk == 0), stop=(kk == 3), skip_group_check=True)
            mean2 = p_lntmp.tile([P, TO], F32, tag="lntmp", name="lntmp")
            nc.scalar.activation(mean2[:, o:o + n], sum_ps[:, o:o + n], AF.Square)
            var = p_lntmp.tile([P, TO], F32, tag="lntmp", name="lntmp")
            nc.vector.scalar_tensor_tensor(var[:, o:o + n], sq_ps[:, o:o + n], 1.0,
                                           mean2[:, o:o + n],
                                           op0=ALU.mult, op1=ALU.subtract)
            std = p_lntmp.tile([P, TO], F32, tag="lntmp", name="lntmp")
            nc.scalar.activation(std[:, o:o + n], var[:, o:o + n], AF.Sqrt,
                                 bias=eps_sb[:, 0:1])
            rstd = p_rstd.tile([P, TO], F16, tag="rstd", name="rstd")
            with nc.allow_low_precision(reason="LN 1/std in fp16 is within tolerance"):
                nc.vector.reciprocal(rstd[:, o:o + n], std[:, o:o + n])
            for m in range(4):
                dx = p_lndx.tile([P, TO], F16, tag="lndx", name="lndx")
                nc.vector.tensor_add(dx[:, o:o + n], xr_l[m][:, o:o + n],
                                     sum_ps[:, o:o + n])
                oap = out_l[m][:, out_off + o:out_off + o + n]
                nc.vector.tensor_mul(oap, dx[:, o:o + n], rstd[:, o:o + n])
                if affcols is not None:
                    gc, bc_ = affcols
                    nc.scalar.activation(oap, out_l[m][:, out_off + o:out_off + o + n],
                                         AF.Identity, bias=par[:, bc_ + m:bc_ + m + 1],
                                         scale=par[:, gc + m:gc + m + 1])

    KCH = ((0, 258), (258, 254))
    for li in range(n_layers):
        last = li == n_layers - 1
        wq = p_wq.tile([P, 2048], F16, tag="wq", name="wq")
        wk = p_wk.tile([P, 2048], F16, tag="wk", name="wk")
        wv = p_wv.tile([P, 2048], F16, tag="wv", name="wv")
        wo = p_wo.tile([P, 2048], F16, tag="wo", name="wo")
        par = p_par.tile([P, 52], F32, tag="par", name="par")
        for t, dn in [(wk, "wk_d"), (wv, "wv_d"), (wq, "wq_d"), (wo, "wo_d"), (par, "par_d")]:
            dma(t[:, :], d[dn][li][:, :])

        bin_kv = dram.tile([C, 1024], F16, tag=f"bkv{li}", name=f"bkv{li}")
        bout_kv = dram.tile([C, 1024], F16, tag=f"bokv{li}", name=f"bokv{li}")

        # Own keys occupy chunks 0..3 of kr_t/vt_t directly (chunk order is
        # arbitrary as long as kr columns match vt rows); the partner half
        # arrives via AllReduce(sum) and subtract on Pool: partner = sum - own.
        with tc.tile_pool(name="psP", bufs=2, space="PSUM") as p_psP:
            # ---- k own-half projection + rope (into kr_t[:, 0:512]) ----
            for m in range(4):
                ps = p_psP.tile([P, 1024], F32, tag="psP", name="psP")
                for (o, n) in KCH:
                    for kk in range(4):
                        mm(ps[:, o:o + n], wk[:, kk * 512 + m * P: kk * 512 + (m + 1) * P],
                           x_t[kk][:, o:o + n], start=(kk == 0), stop=(kk == 3),
                           skip_group_check=True)
                kraw = p_kraw.tile([P, TO], F16, tag="kraw", name="kraw")
                nc.scalar.activation(kraw[:, :], ps[:, 0:TO], AF.Identity,
                                     bias=par[:, 4 + m:5 + m], scale=1.0)
                sh = p_shuf.tile([P, TO], F16, tag="shuf", name="shuf")
                nc.vector.stream_shuffle(sh[:, :], kraw[:, :], SWAP_MASK)
                t1 = p_tmp.tile([P, TO], F16, tag="ropetmp", name="ropetmp")
                t2 = p_tmp.tile([P, TO], F16, tag="ropetmp", name="ropetmp")
                nc.gpsimd.tensor_mul(t1[:, :], kraw[:, :], cos_k[:, :])
                nc.vector.tensor_mul(t2[:, :], sh[:, :], sin_k[:, :])
                nc.vector.tensor_add(kr_t[m][:, 0:TO], t1[:, :], t2[:, :])
                dma(bin_kv[m * P:(m + 1) * P, 0:512], kr_t[m][:, 0:TO])

            # ---- v own-half (transposed; ones blocks pre-set at col 64/head)
            for jj in range(4):
                ps = p_psP.tile([P, 1024], F32, tag="psP", name="psP")
                for kk in range(4):
                    mm(ps[:, 0:512], x_t[kk][:, jj * P:(jj + 1) * P],
                       wv[:, kk * 512: (kk + 1) * 512],
                       start=(kk == 0), stop=(kk == 3))
                vt3 = vt_t[jj][:, :].rearrange("p (h c) -> p h c", c=65)
                ps3 = ps[:, 0:512].rearrange("p (h c) -> p h c", c=64)
                nc.scalar.activation(vt3[:, :, 0:64], ps3[:, :, :], AF.Copy)
                if li == 0:
                    nc.vector.memset(vt3[:, :, 64:65], 1.0)
                    vt3b = vt_t[4 + jj][:, :].rearrange("p (h c) -> p h c", c=65)
                    nc.vector.memset(vt3b[:, :, 64:65], 1.0)
                dma(bin_kv[jj * P:(jj + 1) * P, 512:1024],
                    vt3[:, :, 0:64])

            # ---- k/v exchange: AllReduce(sum) over the pair ----
            if do_gather:
                nc.gpsimd.collective_compute(
                    "AllReduce", ALU.add,
                    replica_groups=[[0, 1], [2, 3], [4, 5], [6, 7]],
                    ins=[bin_kv[:, :].opt()], outs=[bout_kv[:, :].opt()])
                kv_src = bout_kv
            else:
                kv_src = bin_kv
            # ---- q projection + rope (overlaps the exchange) ----
            for m in range(4):
                ps = p_psP.tile([P, 1024], F32, tag="psP", name="psP")
                for kk in range(4):
                    mm(ps[:, 0:512], wq[:, kk * 512 + m * P: kk * 512 + (m + 1) * P],
                       x_t[kk][:, :], start=(kk == 0), stop=(kk == 3))
                qraw = p_qraw.tile([P, TO], F16, tag="qraw", name="qraw")
                nc.scalar.activation(qraw[:, :], ps[:, 0:TO], AF.Identity,
                                     bias=par[:, 0 + m:1 + m], scale=1.0)
                sh = p_shuf.tile([P, TO], F16, tag="shuf", name="shuf")
                nc.vector.stream_shuffle(sh[:, :], qraw[:, :], SWAP_MASK)
                t1 = p_tmp.tile([P, TO], F16, tag="ropetmp", name="ropetmp")
                t2 = p_tmp.tile([P, TO], F16, tag="ropetmp", name="ropetmp")
                nc.gpsimd.tensor_mul(t1[:, :], qraw[:, :], cos_k[:, :])
                nc.vector.tensor_mul(t2[:, :], sh[:, :], sin_k[:, :])
                nc.vector.tensor_add(q_t[m][:, :], t1[:, :], t2[:, :])

            for m in range(4):
                ksum = p_ksum.tile([P, TO], F16, tag="ksum", name="ksum")
                dma(ksum[:, :], kv_src[m * P:(m + 1) * P, 0:512])
                nc.gpsimd.tensor_sub(kr_t[m][:, 512:1024], ksum[:, :], kr_t[m][:, 0:TO])
            for jj in range(4):
                vsum = p_ksum.tile([P, TO], F16, tag="vsum", name="vsum")
                dma(vsum[:, :], kv_src[jj * P:(jj + 1) * P, 512:1024])
                vs3 = vsum[:, :].rearrange("p (h c) -> p h c", c=64)
                va = vt_t[jj][:, :].rearrange("p (h c) -> p h c", c=65)
                vb = vt_t[4 + jj][:, :].rearrange("p (h c) -> p h c", c=65)
                nc.gpsimd.tensor_sub(vb[:, :, 0:64], vs3[:, :, :], va[:, :, 0:64])

        # ---- attention: sc tiles are 1 PSUM bank; deep pipelining ----
        with tc.tile_pool(name="psS", bufs=4, space="PSUM") as p_psS, \
             tc.tile_pool(name="psO", bufs=4, space="PSUM") as p_psO:
            pend = []
            for i in range(4):  # head pairs
                ops_pair = [p_psO.tile([65, TO], F32, tag="psO", name="psO")
                            for _ in range(2)]
                for j in range(8):
                    for sub in range(2):
                        hh = 2 * i + sub
                        o_ps = ops_pair[sub]
                        sc = p_psS.tile([P, TO], F32, tag="psS", name="psS")
                        mm(sc[:, :],
                           kr_t[i][sub * 64:(sub + 1) * 64, j * P:(j + 1) * P],
                           q_t[i][sub * 64:(sub + 1) * 64, :],
                           start=True, stop=True)
                        pt = p_pt.tile([P, TO], F16, tag="pt", name="pt")
                        nc.scalar.activation(pt[:, :], sc[:, :], AF.Exp)
                        mm(o_ps[:, :], vt_t[j][:, hh * 65:(hh + 1) * 65],
                           pt[:, :], start=(j == 0), stop=(j == 7),
                           skip_group_check=True)
                pend.append((i, ops_pair))
                # Normalize is deferred by one pair so its bc matmul never
                # blocks the in-order PE queue at the pair boundary.
                if len(pend) == 2 or i == 3:
                    todo = pend if i == 3 else pend[:1]
                    for (pi, opair) in todo:
                        for sub in range(2):
                            o_ps = opair[sub]
                            nc.vector.tensor_copy(rz_t[sub][64:65, 0:TO],
                                                  o_ps[64:65, 0:TO])
                            bc_ps = p_psS.tile([P, TO], F32, tag="psS", name="psS")
                            mm(bc_ps[0:64, :], ones64[:, :], rz_t[sub][:, 0:TO],
                               start=True, stop=True)
                            rbc = p_rbc.tile([64, TO], F16, tag="rbc", name="rbc")
                            with nc.allow_low_precision(reason="softmax 1/Z in fp16"):
                                nc.vector.reciprocal(rbc[:, :], bc_ps[0:64, :])
                            nc.vector.tensor_mul(onorm_t[pi][sub * 64:(sub + 1) * 64, :],
                                                 o_ps[0:64, :], rbc[:, :])
                            if has_bv:
                                nc.vector.tensor_scalar_add(
                                    onorm_t[pi][sub * 64:(sub + 1) * 64, :],
                                    onorm_t[pi][sub * 64:(sub + 1) * 64, :],
                                    par[sub * 64:(sub + 1) * 64, 48 + pi:49 + pi])
                    pend = pend[1:] if i != 3 else []

        # ---- Wo + residual + LN1 (+ xr-halo exchange and local halo-LN) ----
        with tc.tile_pool(name="psW", bufs=2, space="PSUM") as p_psW, \
             tc.tile_pool(name="psHL", bufs=1, space="PSUM") as p_psHL:
            bin_h = dram.tile([C, 4], F16, tag=f"bh{li}", name=f"bh{li}")
            bout_h = dram.tile([C, 4], F16, tag=f"boh{li}", name=f"boh{li}")
            xr_l = []
            for m in range(4):
                ps = p_psW.tile([P, 1024], F32, tag="psW", name="psW")
                for kk in range(4):
                    mm(ps[:, 0:512], wo[:, kk * 512 + m * P: kk * 512 + (m + 1) * P],
                       onorm_t[kk][:, :], start=(kk == 0), stop=(kk == 3))
                ta = p_qraw.tile([P, TO], F16, tag="qraw", name="qraw")
                nc.scalar.activation(ta[:, :], ps[:, 0:TO], AF.Identity,
                                     bias=par[:, 8 + m:9 + m], scale=1.0)
                xr = p_resid.tile([P, TO], F16, tag="resid", name="resid")
                nc.vector.tensor_add(xr[:, :], ta[:, :], x_t[m][:, :])
                xr_l.append(xr)
                dma(bin_h[m * P:(m + 1) * P, 0:2], xr[:, 0:2])
                dma(bin_h[m * P:(m + 1) * P, 2:4], xr[:, 510:512])
            # exchange the xr boundary columns while LN1 main runs
            if do_gather:
                nc.gpsimd.collective_compute(
                    "AllReduce", ALU.add,
                    replica_groups=[[0, 1], [2, 3], [4, 5], [6, 7]],
                    ins=[bin_h[:, :].opt()], outs=[bout_h[:, :].opt()])
                h_src = bout_h
            else:
                h_src = bin_h
            ln(xr_l, KCH, par, (32, 36) if ln1_aff else None, x1b_t, p_psW, "psW", out_off=2)
            xh_l = []
            for m in range(4):
                hS = p_halo.tile([P, 4], F16, tag="halo", name="halo")
                dma(hS[:, :], h_src[m * P:(m + 1) * P, :])
                xh = p_halo.tile([P, 4], F16, tag="haloxh", name="haloxh")
                # cols 0:2 = left-halo xr (partner.last2); 2:4 = right (first2)
                nc.vector.tensor_sub(xh[:, 0:2], hS[:, 2:4], xr_l[m][:, 510:512])
                nc.vector.tensor_sub(xh[:, 2:4], hS[:, 0:2], xr_l[m][:, 0:2])
                xh_l.append(xh)
            # local LayerNorm of the 4 halo columns
            hps = p_psHL.tile([P, 8], F32, tag="psHL", name="psHL")
            hps2 = p_psHL.tile([P, 8], F32, tag="psHL2", name="psHL2")
            for kk in range(4):
                mm(hps[:, 0:4], onesm[:, 0:128], xh_l[kk][:, :],
                   start=(kk == 0), stop=(kk == 3), skip_group_check=True)
            sqh_l = []
            for kk in range(4):
                sqh = p_halo.tile([P, 4], F16, tag="halosq", name="halosq")
                nc.vector.tensor_mul(sqh[:, :], xh_l[kk][:, :], xh_l[kk][:, :])
                sqh_l.append(sqh)
            for kk in range(4):
                mm(hps2[:, 0:4], onesm[:, 128:256], sqh_l[kk][:, :],
                   start=(kk == 0), stop=(kk == 3), skip_group_check=True)
            m2h = p_halo.tile([P, 4], F32, tag="halof", name="halof")
            nc.scalar.activation(m2h[:, :], hps[:, 0:4], AF.Square)
            varh = p_halo.tile([P, 4], F32, tag="halof", name="halof")
            nc.vector.scalar_tensor_tensor(varh[:, :], hps2[:, 0:4], 1.0, m2h[:, :],
                                           op0=ALU.mult, op1=ALU.subtract)
            stdh = p_halo.tile([P, 4], F32, tag="halof", name="halof")
            nc.scalar.activation(stdh[:, :], varh[:, :], AF.Sqrt, bias=eps_sb[:, 0:1])
            rsh = p_halo.tile([P, 4], F16, tag="halor", name="halor")
            with nc.allow_low_precision(reason="LN 1/std in fp16 is within tolerance"):
                nc.vector.reciprocal(rsh[:, :], stdh[:, :])
            for m in range(4):
                dxh = p_halo.tile([P, 4], F16, tag="halodx", name="halodx")
                nc.vector.tensor_add(dxh[:, :], xh_l[m][:, :], hps[:, 0:4])
                xlh = p_halo.tile([P, 4], F16, tag="halot", name="halot")
                nc.vector.tensor_mul(xlh[:, :], dxh[:, :], rsh[:, :])
                if ln1_aff:
                    nc.scalar.activation(xlh[:, :], xlh[:, :], AF.Identity,
                                         bias=par[:, 36 + m:37 + m],
                                         scale=par[:, 32 + m:33 + m])
                nc.vector.tensor_scalar_mul(x1b_t[m][:, 0:2], xlh[:, 0:2], hcoef[:, 0:1])
                nc.vector.tensor_scalar_mul(x1b_t[m][:, 514:516], xlh[:, 2:4], hcoef[:, 2:3])

        # ---- FFN ----
        # h tile index i = own position i-1 (conv halo via x1b cols 0:2/514:516).
        # Main h columns [2,512) use own x1b only and start right after LN1;
        # the 4 exchange-dependent edge columns accumulate in a separate
        # shared psE tile so the exchange never stalls the fm pipeline.
        with tc.tile_pool(name="psH", bufs=2, space="PSUM") as p_psH, \
             tc.tile_pool(name="psY", bufs=4, space="PSUM") as p_psY:
            y_ps = [p_psY.tile([P, TO], F32, tag="psY", name="psY") for m in range(4)]
            HCH = ((2, 508), (0, 2), (510, 2), (512, 2))
            for fm in range(16):
                w1t = p_w1.tile([P, 12 * 128], F16, tag="w1", name="w1")
                dma(w1t[:, :], d["w1_d"][li][:, fm * 1536:(fm + 1) * 1536])
                h_ps = p_psH.tile([P, 1024], F32, tag="psH", name="psH")
                for (o, n) in HCH:
                    bidx = 0
                    for kk in range(4):
                        for dk in range(3):
                            mm(h_ps[:, o:o + n], w1t[:, bidx * 128:(bidx + 1) * 128],
                               x1b_t[kk][:, dk + o: dk + o + n],
                               start=(bidx == 0), stop=(bidx == 11),
                               skip_group_check=True)
                            bidx += 1
                ht = p_ht.tile([P, 514], F16, tag="ht", name="ht")
                nc.scalar.activation(ht[:, :], h_ps[:, 0:514], AF.Relu,
                                     bias=par[:, 12 + fm:13 + fm], scale=1.0)
                hm = p_hm.tile([P, 514], F16, tag="hm", name="hm")
                nc.vector.tensor_mul(hm[:, :], ht[:, :], maskh[:, :])
                w2t = p_w2.tile([P, 12 * 128], F16, tag="w2", name="w2")
                dma(w2t[:, :], d["w2_d"][li][:, fm * 1536:(fm + 1) * 1536])
                for m in range(4):
                    for dk in range(3):
                        mm(y_ps[m][:, :], w2t[:, (m * 3 + dk) * 128:(m * 3 + dk + 1) * 128],
                           hm[:, dk:dk + 512],
                           start=(fm == 0 and dk == 0), stop=(fm == 15 and dk == 2),
                           skip_group_check=True)
            xr2_l = []
            for m in range(4):
                ta = p_qraw.tile([P, TO], F16, tag="qraw", name="qraw")
                nc.scalar.activation(ta[:, :], y_ps[m][:, :], AF.Identity,
                                     bias=par[:, 28 + m:29 + m], scale=1.0)
                xr2 = p_resid.tile([P, TO], F16, tag="resid", name="resid")
                nc.vector.tensor_add(xr2[:, :], ta[:, :], x1b_t[m][:, 2:2 + TO])
                xr2_l.append(xr2)

        # ---- LN2 (own psum pool so next-layer projections overlap) ----
        with tc.tile_pool(name="psL", bufs=2, space="PSUM") as p_psL:
            if last:
                o32 = [p_out.tile([P, TO], F32, tag=f"o32{m}", name=f"o32{m}")
                       for m in range(4)]
                ln(xr2_l, KCH, par, (40, 44) if ln2_aff else None, o32, p_psL, "psL")
                for m in range(4):
                    dma(d["out_d"][m * P:(m + 1) * P, :], o32[m][:, :])
            else:
                ln(xr2_l, KCH, par, (40, 44) if ln2_aff else None, x_t, p_psL, "psL")

    ctx.close()


def build_program(flags, n_layers=L, do_gather=True):
    nc = bacc.Bacc(target_bir_lowering=False, trn_type="TRN2", num_devices=NC8)
    d = {}
    d["x0_d"] = nc.declare_dram_parameter("x0", [C, TO], F16, isOutput=False)
    d["cos_k_d"] = nc.declare_dram_parameter("cos_k", [128, TO], F16, isOutput=False)
    d["sin_k_d"] = nc.declare_dram_parameter("sin_k", [128, TO], F16, isOutput=False)
    d["maskh_d"] = nc.declare_dram_parameter("maskh", [128, 514], F16, isOutput=False)
    d["hcoef_d"] = nc.declare_dram_parameter("hcoef", [128, 4], F32, isOutput=False)
    d["ones_d"] = nc.declare_dram_parameter("onesmat", [128, 256], F16, isOutput=False)
    for key, shp, dt in [("wq_d", [128, 2048], F16), ("wk_d", [128, 2048], F16),
                         ("wv_d", [128, 2048], F16), ("wo_d", [128, 2048], F16),
                         ("w1_d", [128, 16 * 12 * 128], F16),
                         ("w2_d", [128, 16 * 12 * 128], F16),
                         ("par_d", [128, 52], F32)]:
        d[key] = [nc.declare_dram_parameter(f"{key[:-2]}{i}", shp, dt, isOutput=False)
                  for i in range(L)]
    d["out_d"] = nc.declare_dram_parameter("out", [C, TO], F32, isOutput=True)
    with tile.TileContext(nc) as tc:
        _emit(nc, tc, d, flags, n_layers=n_layers, do_gather=do_gather)
    nc.compile()
    return nc


# ======================= host side =======================

def _rope_tables(tvals):
    theta = 1.0 / (10000.0 ** (np.arange(0, DR, 2) / DR))
    cos = np.ones((128, len(tvals)), np.float32)
    sin = np.zeros((128, len(tvals)), np.float32)
    for r in range(128):
        lc = r % 64
        if lc < 16:
            ang = theta[lc] * tvals
            cos[r] = np.cos(ang); sin[r] = -np.sin(ang)
        elif lc < 32:
            ang = theta[lc - 16] * tvals
            cos[r] = np.cos(ang); sin[r] = np.sin(ang)
    return cos, sin


def _f16(x):
    return np.ascontiguousarray(np.asarray(x, np.float32).astype(np.float16))


def _pack_weights(inputs):
    per_layer = []
    for li in range(L):
        Wq = np.asarray(inputs['Wq'][li][:, :, 0], np.float32) / 8.0
        Wk = np.asarray(inputs['Wk'][li][:, :, 0], np.float32)
        Wv = np.asarray(inputs['Wv'][li][:, :, 0], np.float32)
        Wo = np.asarray(inputs['Wo'][li][:, :, 0], np.float32)
        W1 = np.asarray(inputs['W1'][li], np.float32)  # [F, C, 3]
        W2 = np.asarray(inputs['W2'][li], np.float32)  # [C, F, 3]

        def packT(W):
            WT = W.T
            return np.concatenate([WT[kk * 128:(kk + 1) * 128, :] for kk in range(4)], axis=1)

        wq_p = packT(Wq); wk_p = packT(Wk); wo_p = packT(Wo); wv_p = packT(Wv)
        w1_p = np.zeros((128, 16 * 12 * 128), np.float32)
        for fm in range(16):
            for kk in range(4):
                for dk in range(3):
                    b = kk * 3 + dk
                    w1_p[:, fm * 1536 + b * 128: fm * 1536 + (b + 1) * 128] = \
                        W1[fm * 128:(fm + 1) * 128, kk * 128:(kk + 1) * 128, dk].T
        w2_p = np.zeros((128, 16 * 12 * 128), np.float32)
        for fk in range(16):
            for m in range(4):
                for dk in range(3):
                    b = fk * 12 + m * 3 + dk
                    w2_p[:, b * 128:(b + 1) * 128] = \
                        W2[m * 128:(m + 1) * 128, fk * 128:(fk + 1) * 128, dk].T
        par = np.zeros((128, 52), np.float32)

        def col4(vec):
            return np.asarray(vec, np.float32).reshape(4, 128).T

        par[:, 0:4] = col4(inputs['bq'][li]) / 8.0
        par[:, 4:8] = col4(inputs['bk'][li])
        par[:, 8:12] = col4(inputs['bo'][li])
        par[:, 12:28] = np.asarray(inputs['c1'][li], np.float32).reshape(16, 128).T
        par[:, 28:32] = col4(inputs['c2'][li])
        par[:, 32:36] = col4(inputs['g1'][li])
        par[:, 36:40] = col4(inputs['be1'][li])
        par[:, 40:44] = col4(inputs['g2'][li])
        par[:, 44:48] = col4(inputs['be2'][li])
        par[:, 48:52] = col4(inputs['bv'][li])
        per_layer.append(dict(wq=_f16(wq_p), wk=_f16(wk_p), wv=_f16(wv_p),
                              wo=_f16(wo_p), w1=_f16(w1_p), w2=_f16(w2_p), par=par))
    return per_layer


def kernel(**inputs):
    inputs = {k: np.asarray(v) for k, v in inputs.items()}
    x = inputs['x'].astype(np.float32) * inputs['x_mask'].astype(np.float32)
    has_bv = bool(np.any(inputs['bv'] != 0))
    ln1_aff = bool(np.any(inputs['g1'] != 1) or np.any(inputs['be1'] != 0))
    ln2_aff = bool(np.any(inputs['g2'] != 1) or np.any(inputs['be2'] != 0))
    flags = (has_bv, ln1_aff, ln2_aff)
    if flags not in _CACHE:
        _CACHE[flags] = build_program(flags)
    nc = _CACHE[flags]

    wl = _pack_weights(inputs)
    onesmat = np.concatenate([np.full((128, 128), -1.0 / 512, np.float32),
                              np.full((128, 128), 1.0 / 512, np.float32)], axis=1)

    in_maps = []
    for core in range(NC8):
        g, h = core // 2, core % 2
        t0 = h * TO
        cos_k, sin_k = _rope_tables(np.arange(t0, t0 + TO, dtype=np.float64))
        mh = np.ones((128, 514), np.float32)
        if h == 0:
            mh[:, 0:1] = 0
            hc = np.array([0.0, 0.0, 1.0, 0.0], np.float32)   # cl, -, cr, -
        else:
            mh[:, 513:514] = 0
            hc = np.array([1.0, 0.0, 0.0, 0.0], np.float32)
        im = {
            "x0": _f16(x[g][:, t0:t0 + TO]),
            "cos_k": _f16(cos_k), "sin_k": _f16(sin_k),
            "maskh": _f16(mh),
            "hcoef": np.repeat(hc[None, :], 128, axis=0),
            "onesmat": _f16(onesmat),
        }
        for li in range(L):
            w = wl[li]
            im[f"wq{li}"] = w['wq']; im[f"wk{li}"] = w['wk']
            im[f"wv{li}"] = w['wv']; im[f"wo{li}"] = w['wo']
            im[f"w1{li}"] = w['w1']; im[f"w2{li}"] = w['w2']
            im[f"par{li}"] = w['par']
        in_maps.append(im)

    global LAST_RESULT
    res = run_bass_kernel_spmd(nc, in_maps, core_ids=list(range(NC8)),
                               trace=TRACE)
    LAST_RESULT = res
    out = np.zeros((B, C, T), np.float32)
    for g in range(B):
        out[g, :, 0:TO] = res.results[2 * g]["out"]
        out[g, :, TO:T] = res.results[2 * g + 1]["out"]
    out_dt = np.asarray(inputs['x']).dtype
    return out.astype(out_dt)


# revision 35
# speedup vs baseline: 1.0201x; 1.0201x over previous
# Trainium2 Bass kernel for nn_Encoder (6-layer conv-attention encoder).
# Sharding: 4 batch groups x 2-way sequence split. Each core owns one half of
# one batch element's sequence (512 columns, no halo in the residual). k/v are
# computed for the own half only; the partner half arrives via one fused
# AllReduce(sum) + local subtract per layer (partner = sum - own). The FFN's
# conv halo (4 columns of post-LN1 x1b) is exchanged the same way mid-layer.
# All SBUF data is fp16 (2x/4x DVE modes, better precision than bf16); PSUM
# accumulation stays f32.
import sys
sys.path.insert(0, '/opt/trn_rl_repo')
import numpy as np

from concourse import bacc, tile, mybir
import concourse.bass as bass
from concourse.bass_utils import run_bass_kernel_spmd

B, C, T = 4, 512, 1024
F, KW, L, H = 2048, 3, 6, 8
KC, DR = 64, 32
TO = 512                   # own cols
NC8 = 8
F16 = mybir.dt.float16
F32 = mybir.dt.float32
AF = mybir.ActivationFunctionType
ALU = mybir.AluOpType
EPS = 1e-4
P = 128

_CACHE = {}
TRACE = False
LAST_RESULT = None


def _emit(nc, tc, d, flags, n_layers=L, do_gather=True):
    (has_bv, ln1_aff, ln2_aff) = flags
    from contextlib import ExitStack
    ctx = ExitStack()

    def pool(name, bufs, space="SBUF"):
        return ctx.enter_context(tc.tile_pool(name=name, bufs=bufs, space=space))

    pers = pool("pers", 1)
    dram = pool("dram", 1, space="DRAM")

    p_kraw = pool("kraw", 2)
    p_qraw = pool("qraw", 2)
    p_tmp = pool("ropetmp", 3)
    p_shuf = pool("shuf", 2)
    p_pt = pool("pt", 8)
    p_rbc = pool("rbc", 4)
    p_resid = pool("resid", 10)
    p_lntmp = pool("lntmp", 6)
    p_lndx = pool("lndx", 4)
    p_sq = pool("sq", 8)
    p_rstd = pool("rstd", 2)
    p_ht = pool("ht", 2)
    p_hm = pool("hm", 4)
    p_wq = pool("wq", 2)
    p_wk = pool("wk", 2)
    p_wv = pool("wv", 2)
    p_wo = pool("wo", 2)
    p_w1 = pool("w1", 3)
    p_w2 = pool("w2", 3)
    p_par = pool("par", 2)
    p_out = pool("outp", 1)
    p_halo = pool("halo", 4)
    p_ksum = pool("ksum", 4)

    x_t = [pers.tile([P, TO], F16, tag=f"x{m}", name=f"x{m}") for m in range(4)]
    kr_t = [pers.tile([P, T], F16, tag=f"kr{m}", name=f"kr{m}") for m in range(4)]
    q_t = [pers.tile([P, TO], F16, tag=f"q{m}", name=f"q{m}") for m in range(4)]
    vt_t = [pers.tile([P, 520], F16, tag=f"vt{j}", name=f"vt{j}") for j in range(8)]
    onorm_t = [pers.tile([P, TO], F16, tag=f"on{i}", name=f"on{i}") for i in range(4)]
    rz_t = [pers.tile([65, 520], F16, tag=f"rz{s}", name=f"rz{s}") for s in range(2)]
    ones64 = pers.tile([65, 64], F16, tag="ones64", name="ones64")
    x1b_t = [pers.tile([P, 516], F16, tag=f"x1b{m}", name=f"x1b{m}") for m in range(4)]
    SWAP_MASK = list(range(16, 32)) + list(range(0, 16))
    cos_k = pers.tile([P, TO], F16, tag="cosk", name="cosk")
    sin_k = pers.tile([P, TO], F16, tag="sink", name="sink")
    maskh = pers.tile([P, 514], F16, tag="maskh", name="maskh")
    onesm = pers.tile([P, 256], F16, tag="onesm", name="onesm")
    eps_sb = pers.tile([P, 1], F32, tag="eps", name="eps")
    nexp_sb = pers.tile([P, 1], F32, tag="nexp", name="nexp")
    hcoef = pers.tile([P, 4], F32, tag="hcoef", name="hcoef")

    dma = nc.sync.dma_start
    for name, t in [("cos_k_d", cos_k), ("sin_k_d", sin_k), ("maskh_d", maskh),
                    ("ones_d", onesm), ("hcoef_d", hcoef)]:
        dma(t[:, :], d[name][:, :])
    for m in range(4):
        dma(x_t[m][:, :], d["x0_d"][m * P:(m + 1) * P, :])
    nc.vector.memset(eps_sb[:, :], EPS)
    nc.vector.memset(nexp_sb[:, :], -5.0)
    for s in range(2):
        nc.vector.memset(rz_t[s][0:64, :], 0.0)
    nc.vector.memset(ones64[0:64, :], 0.0)
    nc.vector.memset(ones64[64:65, :], 1.0)

    mm = nc.tensor.matmul

    def ln(xr_l, chunks, par, affcols, out_l, psum_pool, ptag, out_off=0):
        """Channel LayerNorm over 512 columns. xr_l: 4 [P, 512] f16 tiles.
        Writes out_l tiles at column offset out_off. chunks: column ranges
        processed as independent pipelined chains."""
        sum_ps = psum_pool.tile([P, 1024], F32, tag=ptag, name=ptag)
        sq_ps = psum_pool.tile([P, 1024], F32, tag=ptag, name=ptag)
        for (o, n) in chunks:
            for kk in range(4):
                mm(sum_ps[:, o:o + n], onesm[:, 0:128], xr_l[kk][:, o:o + n],
                   start=(kk == 0), stop=(kk == 3), skip_group_check=True)
            sq_l = []
            for kk in range(4):
                sq = p_sq.tile([P, TO], F16, tag="sq", name="sq")
                nc.vector.tensor_mul(sq[:, o:o + n], xr_l[kk][:, o:o + n],
                                     xr_l[kk][:, o:o + n])
                sq_l.append(sq)
            for kk in range(4):
                mm(sq_ps[:, o:o + n], onesm[:, 128:256], sq_l[kk][:, o:o + n],
                   start=(kk == 0), stop=(kk == 3), skip_group_check=True)
            mean2 = p_lntmp.tile([P, TO], F32, tag="lntmp", name="lntmp")
            nc.scalar.activation(mean2[:, o:o + n], sum_ps[:, o:o + n], AF.Square)
            var = p_lntmp.tile([P, TO], F32, tag="lntmp", name="lntmp")
            nc.vector.scalar_tensor_tensor(var[:, o:o + n], sq_ps[:, o:o + n], 1.0,
                                           mean2[:, o:o + n],
                                           op0=ALU.mult, op1=ALU.subtract)
            std = p_lntmp.tile([P, TO], F32, tag="lntmp", name="lntmp")
            nc.scalar.activation(std[:, o:o + n], var[:, o:o + n], AF.Sqrt,
                                 bias=eps_sb[:, 0:1])
            rstd = p_rstd.tile([P, TO], F16, tag="rstd", name="rstd")
            with nc.allow_low_precision(reason="LN 1/std in fp16 is within tolerance"):
                nc.vector.reciprocal(rstd[:, o:o + n], std[:, o:o + n])
            for m in range(4):
                dx = p_lndx.tile([P, TO], F16, tag="lndx", name="lndx")
                nc.vector.tensor_add(dx[:, o:o + n], xr_l[m][:, o:o + n],
                                     sum_ps[:, o:o + n])
                oap = out_l[m][:, out_off + o:out_off + o + n]
                nc.vector.tensor_mul(oap, dx[:, o:o + n], rstd[:, o:o + n])
                if affcols is not None:
                    gc, bc_ = affcols
                    nc.scalar.activation(oap, out_l[m][:, out_off + o:out_off + o + n],
                                         AF.Identity, bias=par[:, bc_ + m:bc_ + m + 1],
                                         scale=par[:, gc + m:gc + m + 1])

    KCH = ((0, 258), (258, 254))
    for li in range(n_layers):
        last = li == n_layers - 1
        wq = p_wq.tile([P, 2048], F16, tag="wq", name="wq")
        wk = p_wk.tile([P, 2048], F16, tag="wk", name="wk")
        wv = p_wv.tile([P, 2048], F16, tag="wv", name="wv")
        wo = p_wo.tile([P, 2048], F16, tag="wo", name="wo")
        par = p_par.tile([P, 52], F32, tag="par", name="par")
        for t, dn in [(wk, "wk_d"), (wv, "wv_d"), (wq, "wq_d"), (wo, "wo_d"), (par, "par_d")]:
            dma(t[:, :], d[dn][li][:, :])

        bin_kv = dram.tile([C, 1024], F16, tag=f"bkv{li}", name=f"bkv{li}")
        bout_kv = dram.tile([C, 1024], F16, tag=f"bokv{li}", name=f"bokv{li}")

        # Own keys occupy chunks 0..3 of kr_t/vt_t directly (chunk order is
        # arbitrary as long as kr columns match vt rows); the partner half
        # arrives via AllReduce(sum) and subtract on Pool: partner = sum - own.
        with tc.tile_pool(name="psP", bufs=2, space="PSUM") as p_psP:
            # ---- k own-half projection + rope (into kr_t[:, 0:512]) ----
            for m in range(4):
                ps = p_psP.tile([P, 1024], F32, tag="psP", name="psP")
                for (o, n) in KCH:
                    for kk in range(4):
                        mm(ps[:, o:o + n], wk[:, kk * 512 + m * P: kk * 512 + (m + 1) * P],
                           x_t[kk][:, o:o + n], start=(kk == 0), stop=(kk == 3),
                           skip_group_check=True)
                kraw = p_kraw.tile([P, TO], F16, tag="kraw", name="kraw")
                nc.scalar.activation(kraw[:, :], ps[:, 0:TO], AF.Identity,
                                     bias=par[:, 4 + m:5 + m], scale=1.0)
                sh = p_shuf.tile([P, TO], F16, tag="shuf", name="shuf")
                nc.vector.stream_shuffle(sh[:, :], kraw[:, :], SWAP_MASK)
                t1 = p_tmp.tile([P, TO], F16, tag="ropetmp", name="ropetmp")
                t2 = p_tmp.tile([P, TO], F16, tag="ropetmp", name="ropetmp")
                nc.gpsimd.tensor_mul(t1[:, :], kraw[:, :], cos_k[:, :])
                nc.vector.tensor_mul(t2[:, :], sh[:, :], sin_k[:, :])
                nc.vector.tensor_add(kr_t[m][:, 0:TO], t1[:, :], t2[:, :])
                dma(bin_kv[m * P:(m + 1) * P, 0:512], kr_t[m][:, 0:TO])

            # ---- v own-half (transposed; ones blocks pre-set at col 64/head)
            for jj in range(4):
                ps = p_psP.tile([P, 1024], F32, tag="psP", name="psP")
                for kk in range(4):
                    mm(ps[:, 0:512], x_t[kk][:, jj * P:(jj + 1) * P],
                       wv[:, kk * 512: (kk + 1) * 512],
                       start=(kk == 0), stop=(kk == 3))
                vt3 = vt_t[jj][:, :].rearrange("p (h c) -> p h c", c=65)
                ps3 = ps[:, 0:512].rearrange("p (h c) -> p h c", c=64)
                nc.scalar.activation(vt3[:, :, 0:64], ps3[:, :, :], AF.Copy)
                if li == 0:
                    nc.vector.memset(vt3[:, :, 64:65], 1.0)
                    vt3b = vt_t[4 + jj][:, :].rearrange("p (h c) -> p h c", c=65)
                    nc.vector.memset(vt3b[:, :, 64:65], 1.0)
                dma(bin_kv[jj * P:(jj + 1) * P, 512:1024],
                    vt3[:, :, 0:64])

            # ---- k/v exchange: AllReduce(sum) over the pair ----
            if do_gather:
                nc.gpsimd.collective_compute(
                    "AllReduce", ALU.add,
                    replica_groups=[[0, 1], [2, 3], [4, 5], [6, 7]],
                    ins=[bin_kv[:, :].opt()], outs=[bout_kv[:, :].opt()])
                kv_src = bout_kv
            else:
                kv_src = bin_kv
            # ---- q projection + rope (overlaps the exchange) ----
            for m in range(4):
                ps = p_psP.tile([P, 1024], F32, tag="psP", name="psP")
                for kk in range(4):
                    mm(ps[:, 0:512], wq[:, kk * 512 + m * P: kk * 512 + (m + 1) * P],
                       x_t[kk][:, :], start=(kk == 0), stop=(kk == 3))
                qraw = p_qraw.tile([P, TO], F16, tag="qraw", name="qraw")
                nc.scalar.activation(qraw[:, :], ps[:, 0:TO], AF.Identity,
                                     bias=par[:, 0 + m:1 + m], scale=1.0)
                sh = p_shuf.tile([P, TO], F16, tag="shuf", name="shuf")
                nc.vector.stream_shuffle(sh[:, :], qraw[:, :], SWAP_MASK)
                t1 = p_tmp.tile([P, TO], F16, tag="ropetmp", name="ropetmp")
                t2 = p_tmp.tile([P, TO], F16, tag="ropetmp", name="ropetmp")
                nc.gpsimd.tensor_mul(t1[:, :], qraw[:, :], cos_k[:, :])
                nc.vector.tensor_mul(t2[:, :], sh[:, :], sin_k[:, :])
                nc.vector.tensor_add(q_t[m][:, :], t1[:, :], t2[:, :])

            for m in range(4):
                ksum = p_ksum.tile([P, TO], F16, tag="ksum", name="ksum")
                dma(ksum[:, :], kv_src[m * P:(m + 1) * P, 0:512])
                nc.gpsimd.tensor_sub(kr_t[m][:, 512:1024], ksum[:, :], kr_t[m][:, 0:TO])
            for jj in range(4):
                vsum = p_ksum.tile([P, TO], F16, tag="vsum", name="vsum")
                dma(vsum[:, :], kv_src[jj * P:(jj + 1) * P, 512:1024])
                vs3 = vsum[:, :].rearrange("p (h c) -> p h c", c=64)
                va = vt_t[jj][:, :].rearrange("p (h c) -> p h c", c=65)
                vb = vt_t[4 + jj][:, :].rearrange("p (h c) -> p h c", c=65)
                nc.gpsimd.tensor_sub(vb[:, :, 0:64], vs3[:, :, :], va[:, :, 0:64])

        # ---- attention: sc tiles are 1 PSUM bank; deep pipelining ----
        with tc.tile_pool(name="psS", bufs=4, space="PSUM") as p_psS, \
             tc.tile_pool(name="psO", bufs=4, space="PSUM") as p_psO:
            pend = []
            for i in range(4):  # head pairs
                ops_pair = [p_psO.tile([65, TO], F32, tag="psO", name="psO")
                            for _ in range(2)]
                for j in range(8):
                    for sub in range(2):
                        hh = 2 * i + sub
                        o_ps = ops_pair[sub]
                        sc = p_psS.tile([P, TO], F32, tag="psS", name="psS")
                        mm(sc[:, :],
                           kr_t[i][sub * 64:(sub + 1) * 64, j * P:(j + 1) * P],
                           q_t[i][sub * 64:(sub + 1) * 64, :],
                           start=True, stop=True)
                        # exp is shifted by -5 so the fp16 Z=sum(exp) row can
                        # never overflow (softmax is shift-invariant)
                        pt = p_pt.tile([P, TO], F16, tag="pt", name="pt")
                        nc.scalar.activation(pt[:, :], sc[:, :], AF.Exp, bias=nexp_sb[:, 0:1])
                        mm(o_ps[:, :], vt_t[j][:, hh * 65:(hh + 1) * 65],
                           pt[:, :], start=(j == 0), stop=(j == 7),
                           skip_group_check=True)
                pend.append((i, ops_pair))
                # Normalize is deferred by one pair so its bc matmul never
                # blocks the in-order PE queue at the pair boundary.
                if len(pend) == 2 or i == 3:
                    todo = pend if i == 3 else pend[:1]
                    for (pi, opair) in todo:
                        for sub in range(2):
                            o_ps = opair[sub]
                            nc.vector.tensor_copy(rz_t[sub][64:65, 0:TO],
                                                  o_ps[64:65, 0:TO])
                            bc_ps = p_psS.tile([P, TO], F32, tag="psS", name="psS")
                            mm(bc_ps[0:64, :], ones64[:, :], rz_t[sub][:, 0:TO],
                               start=True, stop=True)
                            rbc = p_rbc.tile([64, TO], F16, tag="rbc", name="rbc")
                            with nc.allow_low_precision(reason="softmax 1/Z in fp16"):
                                nc.vector.reciprocal(rbc[:, :], bc_ps[0:64, :])
                            nc.vector.tensor_mul(onorm_t[pi][sub * 64:(sub + 1) * 64, :],
                                                 o_ps[0:64, :], rbc[:, :])
                            if has_bv:
                                nc.vector.tensor_scalar_add(
                                    onorm_t[pi][sub * 64:(sub + 1) * 64, :],
                                    onorm_t[pi][sub * 64:(sub + 1) * 64, :],
                                    par[sub * 64:(sub + 1) * 64, 48 + pi:49 + pi])
                    pend = pend[1:] if i != 3 else []

        # ---- Wo + residual + LN1 (+ xr-halo exchange and local halo-LN) ----
        with tc.tile_pool(name="psW", bufs=2, space="PSUM") as p_psW, \
             tc.tile_pool(name="psHL", bufs=1, space="PSUM") as p_psHL:
            bin_h = dram.tile([C, 4], F16, tag=f"bh{li}", name=f"bh{li}")
            bout_h = dram.tile([C, 4], F16, tag=f"boh{li}", name=f"boh{li}")
            xr_l = []
            for m in range(4):
                ps = p_psW.tile([P, 1024], F32, tag="psW", name="psW")
                for kk in range(4):
                    mm(ps[:, 0:512], wo[:, kk * 512 + m * P: kk * 512 + (m + 1) * P],
                       onorm_t[kk][:, :], start=(kk == 0), stop=(kk == 3))
                xr = p_resid.tile([P, TO], F16, tag="resid", name="resid")
                nc.vector.scalar_tensor_tensor(xr[:, :], ps[:, 0:TO], par[:, 8 + m:9 + m],
                                               x_t[m][:, :], op0=ALU.add, op1=ALU.add)
                xr_l.append(xr)
                dma(bin_h[m * P:(m + 1) * P, 0:2], xr[:, 0:2])
                dma(bin_h[m * P:(m + 1) * P, 2:4], xr[:, 510:512])
            # exchange the xr boundary columns while LN1 main runs
            if do_gather:
                nc.gpsimd.collective_compute(
                    "AllReduce", ALU.add,
                    replica_groups=[[0, 1], [2, 3], [4, 5], [6, 7]],
                    ins=[bin_h[:, :].opt()], outs=[bout_h[:, :].opt()])
                h_src = bout_h
            else:
                h_src = bin_h
            ln(xr_l, KCH, par, (32, 36) if ln1_aff else None, x1b_t, p_psW, "psW", out_off=2)
            xh_l = []
            for m in range(4):
                hS = p_halo.tile([P, 4], F16, tag="halo", name="halo")
                dma(hS[:, :], h_src[m * P:(m + 1) * P, :])
                xh = p_halo.tile([P, 4], F16, tag="haloxh", name="haloxh")
                # cols 0:2 = left-halo xr (partner.last2); 2:4 = right (first2)
                nc.vector.tensor_sub(xh[:, 0:2], hS[:, 2:4], xr_l[m][:, 510:512])
                nc.vector.tensor_sub(xh[:, 2:4], hS[:, 0:2], xr_l[m][:, 0:2])
                xh_l.append(xh)
            # local LayerNorm of the 4 halo columns
            hps = p_psHL.tile([P, 8], F32, tag="psHL", name="psHL")
            hps2 = p_psHL.tile([P, 8], F32, tag="psHL2", name="psHL2")
            for kk in range(4):
                mm(hps[:, 0:4], onesm[:, 0:128], xh_l[kk][:, :],
                   start=(kk == 0), stop=(kk == 3), skip_group_check=True)
            sqh_l = []
            for kk in range(4):
                sqh = p_halo.tile([P, 4], F16, tag="halosq", name="halosq")
                nc.vector.tensor_mul(sqh[:, :], xh_l[kk][:, :], xh_l[kk][:, :])
                sqh_l.append(sqh)
            for kk in range(4):
                mm(hps2[:, 0:4], onesm[:, 128:256], sqh_l[kk][:, :],
                   start=(kk == 0), stop=(kk == 3), skip_group_check=True)
            m2h = p_halo.tile([P, 4], F32, tag="halof", name="halof")
            nc.scalar.activation(m2h[:, :], hps[:, 0:4], AF.Square)
            varh = p_halo.tile([P, 4], F32, tag="halof", name="halof")
            nc.vector.scalar_tensor_tensor(varh[:, :], hps2[:, 0:4], 1.0, m2h[:, :],
                                           op0=ALU.mult, op1=ALU.subtract)
            stdh = p_halo.tile([P, 4], F32, tag="halof", name="halof")
            nc.scalar.activation(stdh[:, :], varh[:, :], AF.Sqrt, bias=eps_sb[:, 0:1])
            rsh = p_halo.tile([P, 4], F16, tag="halor", name="halor")
            with nc.allow_low_precision(reason="LN 1/std in fp16 is within tolerance"):
                nc.vector.reciprocal(rsh[:, :], stdh[:, :])
            for m in range(4):
                dxh = p_halo.tile([P, 4], F16, tag="halodx", name="halodx")
                nc.vector.tensor_add(dxh[:, :], xh_l[m][:, :], hps[:, 0:4])
                xlh = p_halo.tile([P, 4], F16, tag="halot", name="halot")
                nc.vector.tensor_mul(xlh[:, :], dxh[:, :], rsh[:, :])
                if ln1_aff:
                    nc.scalar.activation(xlh[:, :], xlh[:, :], AF.Identity,
                                         bias=par[:, 36 + m:37 + m],
                                         scale=par[:, 32 + m:33 + m])
                nc.vector.tensor_scalar_mul(x1b_t[m][:, 0:2], xlh[:, 0:2], hcoef[:, 0:1])
                nc.vector.tensor_scalar_mul(x1b_t[m][:, 514:516], xlh[:, 2:4], hcoef[:, 2:3])

        # ---- FFN ----
        # h tile index i = own position i-1 (conv halo via x1b cols 0:2/514:516).
        # Main h columns [2,512) use own x1b only and start right after LN1;
        # the 4 exchange-dependent edge columns accumulate in a separate
        # shared psE tile so the exchange never stalls the fm pipeline.
        with tc.tile_pool(name="psH", bufs=2, space="PSUM") as p_psH, \
             tc.tile_pool(name="psY", bufs=4, space="PSUM") as p_psY:
            y_ps = [p_psY.tile([P, TO], F32, tag="psY", name="psY") for m in range(4)]
            HCH = ((2, 508), (0, 2), (510, 2), (512, 2))
            for fm in range(16):
                w1t = p_w1.tile([P, 12 * 128], F16, tag="w1", name="w1")
                dma(w1t[:, :], d["w1_d"][li][:, fm * 1536:(fm + 1) * 1536])
                h_ps = p_psH.tile([P, 1024], F32, tag="psH", name="psH")
                for (o, n) in HCH:
                    bidx = 0
                    for kk in range(4):
                        for dk in range(3):
                            mm(h_ps[:, o:o + n], w1t[:, bidx * 128:(bidx + 1) * 128],
                               x1b_t[kk][:, dk + o: dk + o + n],
                               start=(bidx == 0), stop=(bidx == 11),
                               skip_group_check=True)
                            bidx += 1
                ht = p_ht.tile([P, 514], F16, tag="ht", name="ht")
                nc.scalar.activation(ht[:, :], h_ps[:, 0:514], AF.Relu,
                                     bias=par[:, 12 + fm:13 + fm], scale=1.0)
                hm = p_hm.tile([P, 514], F16, tag="hm", name="hm")
                nc.vector.tensor_mul(hm[:, :], ht[:, :], maskh[:, :])
                w2t = p_w2.tile([P, 12 * 128], F16, tag="w2", name="w2")
                dma(w2t[:, :], d["w2_d"][li][:, fm * 1536:(fm + 1) * 1536])
                for m in range(4):
                    for dk in range(3):
                        mm(y_ps[m][:, :], w2t[:, (m * 3 + dk) * 128:(m * 3 + dk + 1) * 128],
                           hm[:, dk:dk + 512],
                           start=(fm == 0 and dk == 0), stop=(fm == 15 and dk == 2),
                           skip_group_check=True)
            xr2_l = []
            for m in range(4):
                xr2 = p_resid.tile([P, TO], F16, tag="resid", name="resid")
                nc.vector.scalar_tensor_tensor(xr2[:, :], y_ps[m][:, :],
                                               par[:, 28 + m:29 + m],
                                               x1b_t[m][:, 2:2 + TO],
                                               op0=ALU.add, op1=ALU.add)
                xr2_l.append(xr2)

        # ---- LN2 (own psum pool so next-layer projections overlap) ----
        with tc.tile_pool(name="psL", bufs=2, space="PSUM") as p_psL:
            if last:
                o32 = [p_out.tile([P, TO], F32, tag=f"o32{m}", name=f"o32{m}")
                       for m in range(4)]
                ln(xr2_l, KCH, par, (40, 44) if ln2_aff else None, o32, p_psL, "psL")
                for m in range(4):
                    dma(d["out_d"][m * P:(m + 1) * P, :], o32[m][:, :])
            else:
                ln(xr2_l, KCH, par, (40, 44) if ln2_aff else None, x_t, p_psL, "psL")

    ctx.close()


def build_program(flags, n_layers=L, do_gather=True):
    nc = bacc.Bacc(target_bir_lowering=False, trn_type="TRN2", num_devices=NC8)
    d = {}
    d["x0_d"] = nc.declare_dram_parameter("x0", [C, TO], F16, isOutput=False)
    d["cos_k_d"] = nc.declare_dram_parameter("cos_k", [128, TO], F16, isOutput=False)
    d["sin_k_d"] = nc.declare_dram_parameter("sin_k", [128, TO], F16, isOutput=False)
    d["maskh_d"] = nc.declare_dram_parameter("maskh", [128, 514], F16, isOutput=False)
    d["hcoef_d"] = nc.declare_dram_parameter("hcoef", [128, 4], F32, isOutput=False)
    d["ones_d"] = nc.declare_dram_parameter("onesmat", [128, 256], F16, isOutput=False)
    for key, shp, dt in [("wq_d", [128, 2048], F16), ("wk_d", [128, 2048], F16),
                         ("wv_d", [128, 2048], F16), ("wo_d", [128, 2048], F16),
                         ("w1_d", [128, 16 * 12 * 128], F16),
                         ("w2_d", [128, 16 * 12 * 128], F16),
                         ("par_d", [128, 52], F32)]:
        d[key] = [nc.declare_dram_parameter(f"{key[:-2]}{i}", shp, dt, isOutput=False)
                  for i in range(L)]
    d["out_d"] = nc.declare_dram_parameter("out", [C, TO], F32, isOutput=True)
    with tile.TileContext(nc) as tc:
        _emit(nc, tc, d, flags, n_layers=n_layers, do_gather=do_gather)
    nc.compile()
    return nc


# ======================= host side =======================

def _rope_tables(tvals):
    theta = 1.0 / (10000.0 ** (np.arange(0, DR, 2) / DR))
    cos = np.ones((128, len(tvals)), np.float32)
    sin = np.zeros((128, len(tvals)), np.float32)
    for r in range(128):
        lc = r % 64
        if lc < 16:
            ang = theta[lc] * tvals
            cos[r] = np.cos(ang); sin[r] = -np.sin(ang)
        elif lc < 32:
            ang = theta[lc - 16] * tvals
            cos[r] = np.cos(ang); sin[r] = np.sin(ang)
    return cos, sin


def _f16(x):
    return np.ascontiguousarray(np.asarray(x, np.float32).astype(np.float16))


def _pack_weights(inputs):
    per_layer = []
    for li in range(L):
        Wq = np.asarray(inputs['Wq'][li][:, :, 0], np.float32) / 8.0
        Wk = np.asarray(inputs['Wk'][li][:, :, 0], np.float32)
        Wv = np.asarray(inputs['Wv'][li][:, :, 0], np.float32)
        Wo = np.asarray(inputs['Wo'][li][:, :, 0], np.float32)
        W1 = np.asarray(inputs['W1'][li], np.float32)  # [F, C, 3]
        W2 = np.asarray(inputs['W2'][li], np.float32)  # [C, F, 3]

        def packT(W):
            WT = W.T
            return np.concatenate([WT[kk * 128:(kk + 1) * 128, :] for kk in range(4)], axis=1)

        wq_p = packT(Wq); wk_p = packT(Wk); wo_p = packT(Wo); wv_p = packT(Wv)
        w1_p = np.zeros((128, 16 * 12 * 128), np.float32)
        for fm in range(16):
            for kk in range(4):
                for dk in range(3):
                    b = kk * 3 + dk
                    w1_p[:, fm * 1536 + b * 128: fm * 1536 + (b + 1) * 128] = \
                        W1[fm * 128:(fm + 1) * 128, kk * 128:(kk + 1) * 128, dk].T
        w2_p = np.zeros((128, 16 * 12 * 128), np.float32)
        for fk in range(16):
            for m in range(4):
                for dk in range(3):
                    b = fk * 12 + m * 3 + dk
                    w2_p[:, b * 128:(b + 1) * 128] = \
                        W2[m * 128:(m + 1) * 128, fk * 128:(fk + 1) * 128, dk].T
        par = np.zeros((128, 52), np.float32)

        def col4(vec):
            return np.asarray(vec, np.float32).reshape(4, 128).T

        par[:, 0:4] = col4(inputs['bq'][li]) / 8.0
        par[:, 4:8] = col4(inputs['bk'][li])
        par[:, 8:12] = col4(inputs['bo'][li])
        par[:, 12:28] = np.asarray(inputs['c1'][li], np.float32).reshape(16, 128).T
        par[:, 28:32] = col4(inputs['c2'][li])
        par[:, 32:36] = col4(inputs['g1'][li])
        par[:, 36:40] = col4(inputs['be1'][li])
        par[:, 40:44] = col4(inputs['g2'][li])
        par[:, 44:48] = col4(inputs['be2'][li])
        par[:, 48:52] = col4(inputs['bv'][li])
        per_layer.append(dict(wq=_f16(wq_p), wk=_f16(wk_p), wv=_f16(wv_p),
                              wo=_f16(wo_p), w1=_f16(w1_p), w2=_f16(w2_p), par=par))
    return per_layer


def kernel(**inputs):
    inputs = {k: np.asarray(v) for k, v in inputs.items()}
    x = inputs['x'].astype(np.float32) * inputs['x_mask'].astype(np.float32)
    has_bv = bool(np.any(inputs['bv'] != 0))
    ln1_aff = bool(np.any(inputs['g1'] != 1) or np.any(inputs['be1'] != 0))
    ln2_aff = bool(np.any(inputs['g2'] != 1) or np.any(inputs['be2'] != 0))
    flags = (has_bv, ln1_aff, ln2_aff)
    if flags not in _CACHE:
        _CACHE[flags] = build_program(flags)
    nc = _CACHE[flags]

    wl = _pack_weights(inputs)
    onesmat = np.concatenate([np.full((128, 128), -1.0 / 512, np.float32),
                              np.full((128, 128), 1.0 / 512, np.float32)], axis=1)

    in_maps = []
    for core in range(NC8):
        g, h = core // 2, core % 2
        t0 = h * TO
        cos_k, sin_k = _rope_tables(np.arange(t0, t0 + TO, dtype=np.float64))
        mh = np.ones((128, 514), np.float32)
        if h == 0:
            mh[:, 0:1] = 0
            hc = np.array([0.0, 0.0, 1.0, 0.0], np.float32)   # cl, -, cr, -
        else:
            mh[:, 513:514] = 0
            hc = np.array([1.0, 0.0, 0.0, 0.0], np.float32)
        im = {
            "x0": _f16(x[g][:, t0:t0 + TO]),
            "cos_k": _f16(cos_k), "sin_k": _f16(sin_k),
            "maskh": _f16(mh),
            "hcoef": np.repeat(hc[None, :], 128, axis=0),
            "onesmat": _f16(onesmat),
        }
        for li in range(L):
            w = wl[li]
            im[f"wq{li}"] = w['wq']; im[f"wk{li}"] = w['wk']
            im[f"wv{li}"] = w['wv']; im[f"wo{li}"] = w['wo']
            im[f"w1{li}"] = w['w1']; im[f"w2{li}"] = w['w2']
            im[f"par{li}"] = w['par']
        in_maps.append(im)

    global LAST_RESULT
    res = run_bass_kernel_spmd(nc, in_maps, core_ids=list(range(NC8)),
                               trace=TRACE)
    LAST_RESULT = res
    out = np.zeros((B, C, T), np.float32)
    for g in range(B):
        out[g, :, 0:TO] = res.results[2 * g]["out"]
        out[g, :, TO:T] = res.results[2 * g + 1]["out"]
    out_dt = np.asarray(inputs['x']).dtype
    return out.astype(out_dt)


# revision 36
# speedup vs baseline: 1.0206x; 1.0005x over previous
# Trainium2 Bass kernel for nn_Encoder (6-layer conv-attention encoder).
# Sharding: 4 batch groups x 2-way sequence split. Each core owns one half of
# one batch element's sequence (512 columns, no halo in the residual). k/v are
# computed for the own half only; the partner half arrives via one fused
# AllReduce(sum) + local subtract per layer (partner = sum - own). The FFN's
# conv halo (4 columns of post-LN1 x1b) is exchanged the same way mid-layer.
# All SBUF data is fp16 (2x/4x DVE modes, better precision than bf16); PSUM
# accumulation stays f32.
import sys
sys.path.insert(0, '/opt/trn_rl_repo')
import numpy as np

from concourse import bacc, tile, mybir
import concourse.bass as bass
from concourse.bass_utils import run_bass_kernel_spmd

B, C, T = 4, 512, 1024
F, KW, L, H = 2048, 3, 6, 8
KC, DR = 64, 32
TO = 512                   # own cols
NC8 = 8
F16 = mybir.dt.float16
F32 = mybir.dt.float32
AF = mybir.ActivationFunctionType
ALU = mybir.AluOpType
EPS = 1e-4
P = 128

_CACHE = {}
TRACE = False
LAST_RESULT = None


def _emit(nc, tc, d, flags, n_layers=L, do_gather=True):
    (has_bv, ln1_aff, ln2_aff) = flags
    from contextlib import ExitStack
    ctx = ExitStack()

    def pool(name, bufs, space="SBUF"):
        return ctx.enter_context(tc.tile_pool(name=name, bufs=bufs, space=space))

    pers = pool("pers", 1)
    dram = pool("dram", 1, space="DRAM")

    p_kraw = pool("kraw", 2)
    p_qraw = pool("qraw", 2)
    p_tmp = pool("ropetmp", 3)
    p_shuf = pool("shuf", 2)
    p_pt = pool("pt", 10)
    p_rbc = pool("rbc", 4)
    p_resid = pool("resid", 10)
    p_lntmp = pool("lntmp", 6)
    p_lndx = pool("lndx", 4)
    p_sq = pool("sq", 8)
    p_rstd = pool("rstd", 2)
    p_ht = pool("ht", 3)
    p_hm = pool("hm", 6)
    p_wq = pool("wq", 2)
    p_wk = pool("wk", 2)
    p_wv = pool("wv", 2)
    p_wo = pool("wo", 2)
    p_w1 = pool("w1", 4)
    p_w2 = pool("w2", 4)
    p_par = pool("par", 2)
    p_out = pool("outp", 1)
    p_halo = pool("halo", 4)
    p_ksum = pool("ksum", 4)

    x_t = [pers.tile([P, TO], F16, tag=f"x{m}", name=f"x{m}") for m in range(4)]
    kr_t = [pers.tile([P, T], F16, tag=f"kr{m}", name=f"kr{m}") for m in range(4)]
    q_t = [pers.tile([P, TO], F16, tag=f"q{m}", name=f"q{m}") for m in range(4)]
    vt_t = [pers.tile([P, 520], F16, tag=f"vt{j}", name=f"vt{j}") for j in range(8)]
    onorm_t = [pers.tile([P, TO], F16, tag=f"on{i}", name=f"on{i}") for i in range(4)]
    rz_t = [pers.tile([65, 520], F16, tag=f"rz{s}", name=f"rz{s}") for s in range(2)]
    ones64 = pers.tile([65, 64], F16, tag="ones64", name="ones64")
    x1b_t = [pers.tile([P, 516], F16, tag=f"x1b{m}", name=f"x1b{m}") for m in range(4)]
    SWAP_MASK = list(range(16, 32)) + list(range(0, 16))
    cos_k = pers.tile([P, TO], F16, tag="cosk", name="cosk")
    sin_k = pers.tile([P, TO], F16, tag="sink", name="sink")
    maskh = pers.tile([P, 514], F16, tag="maskh", name="maskh")
    onesm = pers.tile([P, 256], F16, tag="onesm", name="onesm")
    eps_sb = pers.tile([P, 1], F32, tag="eps", name="eps")
    nexp_sb = pers.tile([P, 1], F32, tag="nexp", name="nexp")
    hcoef = pers.tile([P, 4], F32, tag="hcoef", name="hcoef")

    dma = nc.sync.dma_start
    for name, t in [("cos_k_d", cos_k), ("sin_k_d", sin_k), ("maskh_d", maskh),
                    ("ones_d", onesm), ("hcoef_d", hcoef)]:
        dma(t[:, :], d[name][:, :])
    for m in range(4):
        dma(x_t[m][:, :], d["x0_d"][m * P:(m + 1) * P, :])
    nc.vector.memset(eps_sb[:, :], EPS)
    nc.vector.memset(nexp_sb[:, :], -5.0)
    for s in range(2):
        nc.vector.memset(rz_t[s][0:64, :], 0.0)
    nc.vector.memset(ones64[0:64, :], 0.0)
    nc.vector.memset(ones64[64:65, :], 1.0)

    mm = nc.tensor.matmul

    def ln(xr_l, chunks, par, affcols, out_l, psum_pool, ptag, out_off=0):
        """Channel LayerNorm over 512 columns. xr_l: 4 [P, 512] f16 tiles.
        Writes out_l tiles at column offset out_off. chunks: column ranges
        processed as independent pipelined chains."""
        sum_ps = psum_pool.tile([P, 1024], F32, tag=ptag, name=ptag)
        sq_ps = psum_pool.tile([P, 1024], F32, tag=ptag, name=ptag)
        for (o, n) in chunks:
            for kk in range(4):
                mm(sum_ps[:, o:o + n], onesm[:, 0:128], xr_l[kk][:, o:o + n],
                   start=(kk == 0), stop=(kk == 3), skip_group_check=True)
            sq_l = []
            for kk in range(4):
                sq = p_sq.tile([P, TO], F16, tag="sq", name="sq")
                nc.vector.tensor_mul(sq[:, o:o + n], xr_l[kk][:, o:o + n],
                                     xr_l[kk][:, o:o + n])
                sq_l.append(sq)
            for kk in range(4):
                mm(sq_ps[:, o:o + n], onesm[:, 128:256], sq_l[kk][:, o:o + n],
                   start=(kk == 0), stop=(kk == 3), skip_group_check=True)
            mean2 = p_lntmp.tile([P, TO], F32, tag="lntmp", name="lntmp")
            nc.scalar.activation(mean2[:, o:o + n], sum_ps[:, o:o + n], AF.Square)
            var = p_lntmp.tile([P, TO], F32, tag="lntmp", name="lntmp")
            nc.vector.scalar_tensor_tensor(var[:, o:o + n], sq_ps[:, o:o + n], 1.0,
                                           mean2[:, o:o + n],
                                           op0=ALU.mult, op1=ALU.subtract)
            std = p_lntmp.tile([P, TO], F32, tag="lntmp", name="lntmp")
            nc.scalar.activation(std[:, o:o + n], var[:, o:o + n], AF.Sqrt,
                                 bias=eps_sb[:, 0:1])
            rstd = p_rstd.tile([P, TO], F16, tag="rstd", name="rstd")
            with nc.allow_low_precision(reason="LN 1/std in fp16 is within tolerance"):
                nc.vector.reciprocal(rstd[:, o:o + n], std[:, o:o + n])
            for m in range(4):
                dx = p_lndx.tile([P, TO], F16, tag="lndx", name="lndx")
                nc.vector.tensor_add(dx[:, o:o + n], xr_l[m][:, o:o + n],
                                     sum_ps[:, o:o + n])
                oap = out_l[m][:, out_off + o:out_off + o + n]
                nc.vector.tensor_mul(oap, dx[:, o:o + n], rstd[:, o:o + n])
                if affcols is not None:
                    gc, bc_ = affcols
                    nc.scalar.activation(oap, out_l[m][:, out_off + o:out_off + o + n],
                                         AF.Identity, bias=par[:, bc_ + m:bc_ + m + 1],
                                         scale=par[:, gc + m:gc + m + 1])

    KCH = ((0, 258), (258, 254))
    for li in range(n_layers):
        last = li == n_layers - 1
        wq = p_wq.tile([P, 2048], F16, tag="wq", name="wq")
        wk = p_wk.tile([P, 2048], F16, tag="wk", name="wk")
        wv = p_wv.tile([P, 2048], F16, tag="wv", name="wv")
        wo = p_wo.tile([P, 2048], F16, tag="wo", name="wo")
        par = p_par.tile([P, 52], F32, tag="par", name="par")
        for t, dn in [(wk, "wk_d"), (wv, "wv_d"), (wq, "wq_d"), (wo, "wo_d"), (par, "par_d")]:
            dma(t[:, :], d[dn][li][:, :])

        bin_kv = dram.tile([C, 1024], F16, tag=f"bkv{li}", name=f"bkv{li}")
        bout_kv = dram.tile([C, 1024], F16, tag=f"bokv{li}", name=f"bokv{li}")

        # Own keys occupy chunks 0..3 of kr_t/vt_t directly (chunk order is
        # arbitrary as long as kr columns match vt rows); the partner half
        # arrives via AllReduce(sum) and subtract on Pool: partner = sum - own.
        with tc.tile_pool(name="psP", bufs=2, space="PSUM") as p_psP:
            # ---- k own-half projection + rope (into kr_t[:, 0:512]) ----
            for m in range(4):
                ps = p_psP.tile([P, 1024], F32, tag="psP", name="psP")
                for (o, n) in KCH:
                    for kk in range(4):
                        mm(ps[:, o:o + n], wk[:, kk * 512 + m * P: kk * 512 + (m + 1) * P],
                           x_t[kk][:, o:o + n], start=(kk == 0), stop=(kk == 3),
                           skip_group_check=True)
                kraw = p_kraw.tile([P, TO], F16, tag="kraw", name="kraw")
                nc.scalar.activation(kraw[:, :], ps[:, 0:TO], AF.Identity,
                                     bias=par[:, 4 + m:5 + m], scale=1.0)
                sh = p_shuf.tile([P, TO], F16, tag="shuf", name="shuf")
                nc.vector.stream_shuffle(sh[:, :], kraw[:, :], SWAP_MASK)
                t1 = p_tmp.tile([P, TO], F16, tag="ropetmp", name="ropetmp")
                t2 = p_tmp.tile([P, TO], F16, tag="ropetmp", name="ropetmp")
                nc.gpsimd.tensor_mul(t1[:, :], kraw[:, :], cos_k[:, :])
                nc.vector.tensor_mul(t2[:, :], sh[:, :], sin_k[:, :])
                nc.vector.tensor_add(kr_t[m][:, 0:TO], t1[:, :], t2[:, :])
                dma(bin_kv[m * P:(m + 1) * P, 0:512], kr_t[m][:, 0:TO])

            # ---- v own-half (transposed; ones blocks pre-set at col 64/head)
            for jj in range(4):
                ps = p_psP.tile([P, 1024], F32, tag="psP", name="psP")
                for kk in range(4):
                    mm(ps[:, 0:512], x_t[kk][:, jj * P:(jj + 1) * P],
                       wv[:, kk * 512: (kk + 1) * 512],
                       start=(kk == 0), stop=(kk == 3))
                vt3 = vt_t[jj][:, :].rearrange("p (h c) -> p h c", c=65)
                ps3 = ps[:, 0:512].rearrange("p (h c) -> p h c", c=64)
                nc.scalar.activation(vt3[:, :, 0:64], ps3[:, :, :], AF.Copy)
                if li == 0:
                    nc.vector.memset(vt3[:, :, 64:65], 1.0)
                    vt3b = vt_t[4 + jj][:, :].rearrange("p (h c) -> p h c", c=65)
                    nc.vector.memset(vt3b[:, :, 64:65], 1.0)
                dma(bin_kv[jj * P:(jj + 1) * P, 512:1024],
                    vt3[:, :, 0:64])

            # ---- k/v exchange: AllReduce(sum) over the pair ----
            if do_gather:
                nc.gpsimd.collective_compute(
                    "AllReduce", ALU.add,
                    replica_groups=[[0, 1], [2, 3], [4, 5], [6, 7]],
                    ins=[bin_kv[:, :].opt()], outs=[bout_kv[:, :].opt()])
                kv_src = bout_kv
            else:
                kv_src = bin_kv
            # ---- q projection + rope (overlaps the exchange) ----
            for m in range(4):
                ps = p_psP.tile([P, 1024], F32, tag="psP", name="psP")
                for kk in range(4):
                    mm(ps[:, 0:512], wq[:, kk * 512 + m * P: kk * 512 + (m + 1) * P],
                       x_t[kk][:, :], start=(kk == 0), stop=(kk == 3))
                qraw = p_qraw.tile([P, TO], F16, tag="qraw", name="qraw")
                nc.scalar.activation(qraw[:, :], ps[:, 0:TO], AF.Identity,
                                     bias=par[:, 0 + m:1 + m], scale=1.0)
                sh = p_shuf.tile([P, TO], F16, tag="shuf", name="shuf")
                nc.vector.stream_shuffle(sh[:, :], qraw[:, :], SWAP_MASK)
                t1 = p_tmp.tile([P, TO], F16, tag="ropetmp", name="ropetmp")
                t2 = p_tmp.tile([P, TO], F16, tag="ropetmp", name="ropetmp")
                nc.gpsimd.tensor_mul(t1[:, :], qraw[:, :], cos_k[:, :])
                nc.vector.tensor_mul(t2[:, :], sh[:, :], sin_k[:, :])
                nc.vector.tensor_add(q_t[m][:, :], t1[:, :], t2[:, :])

            for m in range(4):
                ksum = p_ksum.tile([P, TO], F16, tag="ksum", name="ksum")
                dma(ksum[:, :], kv_src[m * P:(m + 1) * P, 0:512])
                nc.gpsimd.tensor_sub(kr_t[m][:, 512:1024], ksum[:, :], kr_t[m][:, 0:TO])
            for jj in range(4):
                vsum = p_ksum.tile([P, TO], F16, tag="vsum", name="vsum")
                dma(vsum[:, :], kv_src[jj * P:(jj + 1) * P, 512:1024])
                vs3 = vsum[:, :].rearrange("p (h c) -> p h c", c=64)
                va = vt_t[jj][:, :].rearrange("p (h c) -> p h c", c=65)
                vb = vt_t[4 + jj][:, :].rearrange("p (h c) -> p h c", c=65)
                nc.gpsimd.tensor_sub(vb[:, :, 0:64], vs3[:, :, :], va[:, :, 0:64])

        # ---- attention: sc tiles are 1 PSUM bank; deep pipelining ----
        with tc.tile_pool(name="psS", bufs=4, space="PSUM") as p_psS, \
             tc.tile_pool(name="psO", bufs=4, space="PSUM") as p_psO:
            pend = []
            for i in range(4):  # head pairs
                ops_pair = [p_psO.tile([65, TO], F32, tag="psO", name="psO")
                            for _ in range(2)]
                for j in range(8):
                    for sub in range(2):
                        hh = 2 * i + sub
                        o_ps = ops_pair[sub]
                        sc = p_psS.tile([P, TO], F32, tag="psS", name="psS")
                        mm(sc[:, :],
                           kr_t[i][sub * 64:(sub + 1) * 64, j * P:(j + 1) * P],
                           q_t[i][sub * 64:(sub + 1) * 64, :],
                           start=True, stop=True)
                        # exp is shifted by -5 so the fp16 Z=sum(exp) row can
                        # never overflow (softmax is shift-invariant)
                        pt = p_pt.tile([P, TO], F16, tag="pt", name="pt")
                        nc.scalar.activation(pt[:, :], sc[:, :], AF.Exp, bias=nexp_sb[:, 0:1])
                        mm(o_ps[:, :], vt_t[j][:, hh * 65:(hh + 1) * 65],
                           pt[:, :], start=(j == 0), stop=(j == 7),
                           skip_group_check=True)
                pend.append((i, ops_pair))
                # Normalize is deferred by one pair so its bc matmul never
                # blocks the in-order PE queue at the pair boundary.
                if len(pend) == 2 or i == 3:
                    todo = pend if i == 3 else pend[:1]
                    for (pi, opair) in todo:
                        for sub in range(2):
                            o_ps = opair[sub]
                            nc.vector.tensor_copy(rz_t[sub][64:65, 0:TO],
                                                  o_ps[64:65, 0:TO])
                            bc_ps = p_psS.tile([P, TO], F32, tag="psS", name="psS")
                            mm(bc_ps[0:64, :], ones64[:, :], rz_t[sub][:, 0:TO],
                               start=True, stop=True)
                            rbc = p_rbc.tile([64, TO], F16, tag="rbc", name="rbc")
                            with nc.allow_low_precision(reason="softmax 1/Z in fp16"):
                                nc.vector.reciprocal(rbc[:, :], bc_ps[0:64, :])
                            nc.vector.tensor_mul(onorm_t[pi][sub * 64:(sub + 1) * 64, :],
                                                 o_ps[0:64, :], rbc[:, :])
                            if has_bv:
                                nc.vector.tensor_scalar_add(
                                    onorm_t[pi][sub * 64:(sub + 1) * 64, :],
                                    onorm_t[pi][sub * 64:(sub + 1) * 64, :],
                                    par[sub * 64:(sub + 1) * 64, 48 + pi:49 + pi])
                    pend = pend[1:] if i != 3 else []

        # ---- Wo + residual + LN1 (+ xr-halo exchange and local halo-LN) ----
        with tc.tile_pool(name="psW", bufs=2, space="PSUM") as p_psW, \
             tc.tile_pool(name="psHL", bufs=1, space="PSUM") as p_psHL:
            bin_h = dram.tile([C, 4], F16, tag=f"bh{li}", name=f"bh{li}")
            bout_h = dram.tile([C, 4], F16, tag=f"boh{li}", name=f"boh{li}")
            xr_l = []
            for m in range(4):
                ps = p_psW.tile([P, 1024], F32, tag="psW", name="psW")
                for kk in range(4):
                    mm(ps[:, 0:512], wo[:, kk * 512 + m * P: kk * 512 + (m + 1) * P],
                       onorm_t[kk][:, :], start=(kk == 0), stop=(kk == 3))
                xr = p_resid.tile([P, TO], F16, tag="resid", name="resid")
                nc.vector.scalar_tensor_tensor(xr[:, :], ps[:, 0:TO], par[:, 8 + m:9 + m],
                                               x_t[m][:, :], op0=ALU.add, op1=ALU.add)
                xr_l.append(xr)
                dma(bin_h[m * P:(m + 1) * P, 0:2], xr[:, 0:2])
                dma(bin_h[m * P:(m + 1) * P, 2:4], xr[:, 510:512])
            # exchange the xr boundary columns while LN1 main runs
            if do_gather:
                nc.gpsimd.collective_compute(
                    "AllReduce", ALU.add,
                    replica_groups=[[0, 1], [2, 3], [4, 5], [6, 7]],
                    ins=[bin_h[:, :].opt()], outs=[bout_h[:, :].opt()])
                h_src = bout_h
            else:
                h_src = bin_h
            ln(xr_l, KCH, par, (32, 36) if ln1_aff else None, x1b_t, p_psW, "psW", out_off=2)
            xh_l = []
            for m in range(4):
                hS = p_halo.tile([P, 4], F16, tag="halo", name="halo")
                dma(hS[:, :], h_src[m * P:(m + 1) * P, :])
                xh = p_halo.tile([P, 4], F16, tag="haloxh", name="haloxh")
                # cols 0:2 = left-halo xr (partner.last2); 2:4 = right (first2)
                nc.vector.tensor_sub(xh[:, 0:2], hS[:, 2:4], xr_l[m][:, 510:512])
                nc.vector.tensor_sub(xh[:, 2:4], hS[:, 0:2], xr_l[m][:, 0:2])
                xh_l.append(xh)
            # local LayerNorm of the 4 halo columns
            hps = p_psHL.tile([P, 8], F32, tag="psHL", name="psHL")
            hps2 = p_psHL.tile([P, 8], F32, tag="psHL2", name="psHL2")
            for kk in range(4):
                mm(hps[:, 0:4], onesm[:, 0:128], xh_l[kk][:, :],
                   start=(kk == 0), stop=(kk == 3), skip_group_check=True)
            sqh_l = []
            for kk in range(4):
                sqh = p_halo.tile([P, 4], F16, tag="halosq", name="halosq")
                nc.vector.tensor_mul(sqh[:, :], xh_l[kk][:, :], xh_l[kk][:, :])
                sqh_l.append(sqh)
            for kk in range(4):
                mm(hps2[:, 0:4], onesm[:, 128:256], sqh_l[kk][:, :],
                   start=(kk == 0), stop=(kk == 3), skip_group_check=True)
            m2h = p_halo.tile([P, 4], F32, tag="halof", name="halof")
            nc.scalar.activation(m2h[:, :], hps[:, 0:4], AF.Square)
            varh = p_halo.tile([P, 4], F32, tag="halof", name="halof")
            nc.vector.scalar_tensor_tensor(varh[:, :], hps2[:, 0:4], 1.0, m2h[:, :],
                                           op0=ALU.mult, op1=ALU.subtract)
            stdh = p_halo.tile([P, 4], F32, tag="halof", name="halof")
            nc.scalar.activation(stdh[:, :], varh[:, :], AF.Sqrt, bias=eps_sb[:, 0:1])
            rsh = p_halo.tile([P, 4], F16, tag="halor", name="halor")
            with nc.allow_low_precision(reason="LN 1/std in fp16 is within tolerance"):
                nc.vector.reciprocal(rsh[:, :], stdh[:, :])
            for m in range(4):
                dxh = p_halo.tile([P, 4], F16, tag="halodx", name="halodx")
                nc.vector.tensor_add(dxh[:, :], xh_l[m][:, :], hps[:, 0:4])
                xlh = p_halo.tile([P, 4], F16, tag="halot", name="halot")
                nc.vector.tensor_mul(xlh[:, :], dxh[:, :], rsh[:, :])
                if ln1_aff:
                    nc.scalar.activation(xlh[:, :], xlh[:, :], AF.Identity,
                                         bias=par[:, 36 + m:37 + m],
                                         scale=par[:, 32 + m:33 + m])
                nc.vector.tensor_scalar_mul(x1b_t[m][:, 0:2], xlh[:, 0:2], hcoef[:, 0:1])
                nc.vector.tensor_scalar_mul(x1b_t[m][:, 514:516], xlh[:, 2:4], hcoef[:, 2:3])

        # ---- FFN ----
        # h tile index i = own position i-1 (conv halo via x1b cols 0:2/514:516).
        # Main h columns [2,512) use own x1b only and start right after LN1;
        # the 4 exchange-dependent edge columns accumulate in a separate
        # shared psE tile so the exchange never stalls the fm pipeline.
        with tc.tile_pool(name="psH", bufs=2, space="PSUM") as p_psH, \
             tc.tile_pool(name="psY", bufs=4, space="PSUM") as p_psY:
            y_ps = [p_psY.tile([P, TO], F32, tag="psY", name="psY") for m in range(4)]
            HCH = ((2, 508), (0, 2), (510, 2), (512, 2))
            for fm in range(16):
                w1t = p_w1.tile([P, 12 * 128], F16, tag="w1", name="w1")
                dma(w1t[:, :], d["w1_d"][li][:, fm * 1536:(fm + 1) * 1536])
                h_ps = p_psH.tile([P, 1024], F32, tag="psH", name="psH")
                for (o, n) in HCH:
                    bidx = 0
                    for kk in range(4):
                        for dk in range(3):
                            mm(h_ps[:, o:o + n], w1t[:, bidx * 128:(bidx + 1) * 128],
                               x1b_t[kk][:, dk + o: dk + o + n],
                               start=(bidx == 0), stop=(bidx == 11),
                               skip_group_check=True)
                            bidx += 1
                ht = p_ht.tile([P, 514], F16, tag="ht", name="ht")
                nc.scalar.activation(ht[:, :], h_ps[:, 0:514], AF.Relu,
                                     bias=par[:, 12 + fm:13 + fm], scale=1.0)
                hm = p_hm.tile([P, 514], F16, tag="hm", name="hm")
                nc.vector.tensor_mul(hm[:, :], ht[:, :], maskh[:, :])
                w2t = p_w2.tile([P, 12 * 128], F16, tag="w2", name="w2")
                dma(w2t[:, :], d["w2_d"][li][:, fm * 1536:(fm + 1) * 1536])
                for m in range(4):
                    for dk in range(3):
                        mm(y_ps[m][:, :], w2t[:, (m * 3 + dk) * 128:(m * 3 + dk + 1) * 128],
                           hm[:, dk:dk + 512],
                           start=(fm == 0 and dk == 0), stop=(fm == 15 and dk == 2),
                           skip_group_check=True)
            xr2_l = []
            for m in range(4):
                xr2 = p_resid.tile([P, TO], F16, tag="resid", name="resid")
                nc.vector.scalar_tensor_tensor(xr2[:, :], y_ps[m][:, :],
                                               par[:, 28 + m:29 + m],
                                               x1b_t[m][:, 2:2 + TO],
                                               op0=ALU.add, op1=ALU.add)
                xr2_l.append(xr2)

        # ---- LN2 (own psum pool so next-layer projections overlap) ----
        with tc.tile_pool(name="psL", bufs=2, space="PSUM") as p_psL:
            if last:
                o32 = [p_out.tile([P, TO], F32, tag=f"o32{m}", name=f"o32{m}")
                       for m in range(4)]
                ln(xr2_l, KCH, par, (40, 44) if ln2_aff else None, o32, p_psL, "psL")
                for m in range(4):
                    dma(d["out_d"][m * P:(m + 1) * P, :], o32[m][:, :])
            else:
                ln(xr2_l, KCH, par, (40, 44) if ln2_aff else None, x_t, p_psL, "psL")

    ctx.close()


def build_program(flags, n_layers=L, do_gather=True):
    nc = bacc.Bacc(target_bir_lowering=False, trn_type="TRN2", num_devices=NC8)
    d = {}
    d["x0_d"] = nc.declare_dram_parameter("x0", [C, TO], F16, isOutput=False)
    d["cos_k_d"] = nc.declare_dram_parameter("cos_k", [128, TO], F16, isOutput=False)
    d["sin_k_d"] = nc.declare_dram_parameter("sin_k", [128, TO], F16, isOutput=False)
    d["maskh_d"] = nc.declare_dram_parameter("maskh", [128, 514], F16, isOutput=False)
    d["hcoef_d"] = nc.declare_dram_parameter("hcoef", [128, 4], F32, isOutput=False)
    d["ones_d"] = nc.declare_dram_parameter("onesmat", [128, 256], F16, isOutput=False)
    for key, shp, dt in [("wq_d", [128, 2048], F16), ("wk_d", [128, 2048], F16),
                         ("wv_d", [128, 2048], F16), ("wo_d", [128, 2048], F16),
                         ("w1_d", [128, 16 * 12 * 128], F16),
                         ("w2_d", [128, 16 * 12 * 128], F16),
                         ("par_d", [128, 52], F32)]:
        d[key] = [nc.declare_dram_parameter(f"{key[:-2]}{i}", shp, dt, isOutput=False)
                  for i in range(L)]
    d["out_d"] = nc.declare_dram_parameter("out", [C, TO], F32, isOutput=True)
    with tile.TileContext(nc) as tc:
        _emit(nc, tc, d, flags, n_layers=n_layers, do_gather=do_gather)
    nc.compile()
    return nc


# ======================= host side =======================

def _rope_tables(tvals):
    theta = 1.0 / (10000.0 ** (np.arange(0, DR, 2) / DR))
    cos = np.ones((128, len(tvals)), np.float32)
    sin = np.zeros((128, len(tvals)), np.float32)
    for r in range(128):
        lc = r % 64
        if lc < 16:
            ang = theta[lc] * tvals
            cos[r] = np.cos(ang); sin[r] = -np.sin(ang)
        elif lc < 32:
            ang = theta[lc - 16] * tvals
            cos[r] = np.cos(ang); sin[r] = np.sin(ang)
    return cos, sin


def _f16(x):
    return np.ascontiguousarray(np.asarray(x, np.float32).astype(np.float16))


def _pack_weights(inputs):
    per_layer = []
    for li in range(L):
        Wq = np.asarray(inputs['Wq'][li][:, :, 0], np.float32) / 8.0
        Wk = np.asarray(inputs['Wk'][li][:, :, 0], np.float32)
        Wv = np.asarray(inputs['Wv'][li][:, :, 0], np.float32)
        Wo = np.asarray(inputs['Wo'][li][:, :, 0], np.float32)
        W1 = np.asarray(inputs['W1'][li], np.float32)  # [F, C, 3]
        W2 = np.asarray(inputs['W2'][li], np.float32)  # [C, F, 3]

        def packT(W):
            WT = W.T
            return np.concatenate([WT[kk * 128:(kk + 1) * 128, :] for kk in range(4)], axis=1)

        wq_p = packT(Wq); wk_p = packT(Wk); wo_p = packT(Wo); wv_p = packT(Wv)
        w1_p = np.zeros((128, 16 * 12 * 128), np.float32)
        for fm in range(16):
            for kk in range(4):
                for dk in range(3):
                    b = kk * 3 + dk
                    w1_p[:, fm * 1536 + b * 128: fm * 1536 + (b + 1) * 128] = \
                        W1[fm * 128:(fm + 1) * 128, kk * 128:(kk + 1) * 128, dk].T
        w2_p = np.zeros((128, 16 * 12 * 128), np.float32)
        for fk in range(16):
            for m in range(4):
                for dk in range(3):
                    b = fk * 12 + m * 3 + dk
                    w2_p[:, b * 128:(b + 1) * 128] = \
                        W2[m * 128:(m + 1) * 128, fk * 128:(fk + 1) * 128, dk].T
        par = np.zeros((128, 52), np.float32)

        def col4(vec):
            return np.asarray(vec, np.float32).reshape(4, 128).T

        par[:, 0:4] = col4(inputs['bq'][li]) / 8.0
        par[:, 4:8] = col4(inputs['bk'][li])
        par[:, 8:12] = col4(inputs['bo'][li])
        par[:, 12:28] = np.asarray(inputs['c1'][li], np.float32).reshape(16, 128).T
        par[:, 28:32] = col4(inputs['c2'][li])
        par[:, 32:36] = col4(inputs['g1'][li])
        par[:, 36:40] = col4(inputs['be1'][li])
        par[:, 40:44] = col4(inputs['g2'][li])
        par[:, 44:48] = col4(inputs['be2'][li])
        par[:, 48:52] = col4(inputs['bv'][li])
        per_layer.append(dict(wq=_f16(wq_p), wk=_f16(wk_p), wv=_f16(wv_p),
                              wo=_f16(wo_p), w1=_f16(w1_p), w2=_f16(w2_p), par=par))
    return per_layer


def kernel(**inputs):
    inputs = {k: np.asarray(v) for k, v in inputs.items()}
    x = inputs['x'].astype(np.float32) * inputs['x_mask'].astype(np.float32)
    has_bv = bool(np.any(inputs['bv'] != 0))
    ln1_aff = bool(np.any(inputs['g1'] != 1) or np.any(inputs['be1'] != 0))
    ln2_aff = bool(np.any(inputs['g2'] != 1) or np.any(inputs['be2'] != 0))
    flags = (has_bv, ln1_aff, ln2_aff)
    if flags not in _CACHE:
        _CACHE[flags] = build_program(flags)
    nc = _CACHE[flags]

    wl = _pack_weights(inputs)
    onesmat = np.concatenate([np.full((128, 128), -1.0 / 512, np.float32),
                              np.full((128, 128), 1.0 / 512, np.float32)], axis=1)

    in_maps = []
    for core in range(NC8):
        g, h = core // 2, core % 2
        t0 = h * TO
        cos_k, sin_k = _rope_tables(np.arange(t0, t0 + TO, dtype=np.float64))
        mh = np.ones((128, 514), np.float32)
        if h == 0:
            mh[:, 0:1] = 0
            hc = np.array([0.0, 0.0, 1.0, 0.0], np.float32)   # cl, -, cr, -
        else:
            mh[:, 513:514] = 0
            hc = np.array([1.0, 0.0, 0.0, 0.0], np.float32)
        im = {
            "x0": _f16(x[g][:, t0:t0 + TO]),
            "cos_k": _f16(cos_k), "sin_k": _f16(sin_k),
            "maskh": _f16(mh),
            "hcoef": np.repeat(hc[None, :], 128, axis=0),
            "onesmat": _f16(onesmat),
        }
        for li in range(L):
            w = wl[li]
            im[f"wq{li}"] = w['wq']; im[f"wk{li}"] = w['wk']
            im[f"wv{li}"] = w['wv']; im[f"wo{li}"] = w['wo']
            im[f"w1{li}"] = w['w1']; im[f"w2{li}"] = w['w2']
            im[f"par{li}"] = w['par']
        in_maps.append(im)

    global LAST_RESULT
    res = run_bass_kernel_spmd(nc, in_maps, core_ids=list(range(NC8)),
                               trace=TRACE)
    LAST_RESULT = res
    out = np.zeros((B, C, T), np.float32)
    for g in range(B):
        out[g, :, 0:TO] = res.results[2 * g]["out"]
        out[g, :, TO:T] = res.results[2 * g + 1]["out"]
    out_dt = np.asarray(inputs['x']).dtype
    return out.astype(out_dt)
